# revision 1
# baseline (speedup 1.0000x reference)
"""Trainium2 Bass kernel for DebertaV3+CRF token-classification loss.

Computes: LayerNorm -> Linear(1024,512) -> GELU(exact) -> Linear(512,9)
-> CRF negative log-likelihood (mean over batch).

Strategy: data-parallel over batch across 8 NeuronCores (8 examples each).
The CRF partition function is computed as a balanced tree of 9x9 matrix
products in probability domain (log-rescaled from level 3 on), which turns
the 511-step sequential scan into ~9 parallel levels of elementwise
multiply+reduce on the vector engine. mm2 emits emissions with consecutive
token PAIRS on one partition so tree level 1 runs directly in SBUF.

Self-contained: only imports numpy/ml_dtypes and the system concourse repo.
"""

import sys

for _p in ("/opt/trn_rl_repo", "/root/.axon_site/_ro/trn_rl_repo"):
    if _p not in sys.path:
        sys.path.append(_p)

import numpy as np
import ml_dtypes

import concourse.bass as bass
import concourse.tile as tile
import concourse.mybir as mybir
from concourse.alu_op_type import AluOpType
from bass_rust import AP as RAP, ScopedClock

BF16 = mybir.dt.bfloat16
F32 = mybir.dt.float32
AX = mybir.AxisListType
AF = mybir.ActivationFunctionType
nbf16 = ml_dtypes.bfloat16

B, S, H, L = 64, 512, 1024, 9
EPS = 1e-5
NCORES = 8
BPC = B // NCORES          # examples per core
T = BPC * S                # tokens per core (4096)
NG = BPC                   # token groups of 512 = one example each
KS = H // 128              # 8 k-slices
MS = 512 // 128            # 4 ch-slices
L2 = L * L                 # 81
RESCALE_FROM = 3           # rescale tree levels >= this
NLV = 9                    # tree levels (512 tokens/example)

# lm_all packing: level -> list of (col, row0) per tile
LM_SLOTS = {
    3: [(0, 0), (1, 0), (2, 0), (3, 0)],
    4: [(4, 0), (5, 0)],
    5: [(6, 0)],
    6: [(7, 0)],
    7: [(7, 64)],
    8: [(7, 96)],
    9: [(8, 0)],
}


# ---------------------------------------------------------------------------
# TileContext drain patch: this walrus build rejects >1 semaphore wait on the
# final SP drain ("Too many sync wait commands"); split waits across nops.
def _patched_drain_and_barrier(self, tick_clock, wait_clock):
    drain_inst = self.nc.sync.drain()
    wait_clock.add_sem_waits(
        drain_inst.ins, ScopedClock({None: tick_clock.global_clock})
    )
    si = drain_inst.ins.sync_info
    waits = list(si.on_wait) if si and si.on_wait else []
    if len(waits) > 1:
        si.on_wait = []
        insts = self.nc.cur_bb.bb.instructions
        assert insts[-1] is drain_inst.ins
        insts.pop()
        for w in waits:
            nop = self.nc.sync.nop(nofuse=True)
            nsi = nop.ins.sync_info
            if nsi is None:
                nop.ins.sync_info = mybir.SyncInfo(on_wait=[w], on_update=[])
            else:
                nsi.on_wait = [w]
        insts.append(drain_inst.ins)
    self.nc.all_engine_barrier()
    assert self.sems is not None
    popped = self.nc._tile_sem_poison_stack.pop()
    assert popped is self._sem_poison
    self.nc.clear_and_free_semaphores(list(self.sems.allocated().values()))
    self.nc.all_engine_barrier()


tile.TileContext._drain_and_barrier = _patched_drain_and_barrier


def _split_waits(nc, maxw=1):
    """This walrus build rejects instructions with more than ~1-2 semaphore
    waits; hoist extras onto same-engine nops inserted before the instruction."""
    for f in nc.m.functions:
        for bb in f.blocks:
            insts = bb.instructions
            new = []
            changed = False
            for inst in list(insts):
                si = inst.sync_info
                waits = list(si.on_wait) if si and si.on_wait else []
                if len(waits) > maxw:
                    changed = True
                    si.on_wait = waits[-maxw:]
                    for w in waits[:-maxw]:
                        nop = nc.engines[inst.engine].nop(nofuse=True)
                        cb = nc.cur_bb.bb.instructions
                        assert cb[-1] is nop.ins
                        cb.pop()
                        if nop.ins.sync_info is None:
                            nop.ins.sync_info = mybir.SyncInfo(
                                on_wait=[w], on_update=[])
                        else:
                            nop.ins.sync_info.on_wait = [w]
                        new.append(nop.ins)
                new.append(inst)
            if changed:
                while len(insts):
                    insts.pop()
                for i in new:
                    insts.append(i)


def _pairmul(nc, out_ap, base, rows, engine=None):
    """out[p,i,j] = sum_k A[p,i,k]*B[p,k,j] first half: the multiply.
    base: AP of a [rows, 162] tile region (A | B). Returns nothing."""
    p_ent = list(base.ap[0])
    a_ap = RAP(base.tensor, base.offset, [p_ent, [L, L], [0, L], [1, L]])
    b_ap = RAP(base.tensor, base.offset + L2, [p_ent, [0, L], [1, L], [L, L]])
    eng = engine or nc.vector
    eng.tensor_tensor(out_ap, a_ap, b_ap, op=AluOpType.mult)


# ---------------------------------------------------------------------------
def build_body(tc, reps=1, phases=('load','stats','ln','tr','mm','em','tree')):
    nc = tc.nc
    x_d = nc.dram_tensor("x", [T, H], BF16, kind="ExternalInput").ap()
    w1_d = nc.dram_tensor("w1", [128, KS * MS * 128], BF16,
                          kind="ExternalInput").ap()
    w2_d = nc.dram_tensor("w2", [128, MS * L], BF16, kind="ExternalInput").ap()
    b1_d = nc.dram_tensor("b1", [128, MS], F32, kind="ExternalInput").ap()
    t9_d = nc.dram_tensor("t9", [128, L2], BF16, kind="ExternalInput").ap()
    i81_d = nc.dram_tensor("i81", [1, L2], BF16, kind="ExternalInput").ap()
    oh_d = nc.dram_tensor("oh", [64, NG * 72], F32, kind="ExternalInput").ap()

    om_d = nc.dram_tensor("out_m", [BPC, L2], F32, kind="ExternalOutput").ap()
    oe_d = nc.dram_tensor("out_em0", [1, NG * L], F32,
                          kind="ExternalOutput").ap()
    on_d = nc.dram_tensor("out_ne", [1, NG], F32, kind="ExternalOutput").ap()
    lm_d = nc.dram_tensor("out_lm", [128, 9], F32, kind="ExternalOutput").ap()

    from contextlib import ExitStack
    ctx = tc._build_ctx = ExitStack()
    ctx.__enter__()

    const = ctx.enter_context(tc.tile_pool(name="const", bufs=1))
    xpool = ctx.enter_context(tc.tile_pool(name="xp", bufs=3))
    jpool = ctx.enter_context(tc.tile_pool(name="junk", bufs=2))
    stp = ctx.enter_context(tc.tile_pool(name="stats", bufs=8))
    xnp = ctx.enter_context(tc.tile_pool(name="xn", bufs=3))
    xntp = ctx.enter_context(tc.tile_pool(name="xnt", bufs=2))
    hpool = ctx.enter_context(tc.tile_pool(name="h", bufs=2))
    j3p = ctx.enter_context(tc.tile_pool(name="junk3", bufs=2))
    e4p = ctx.enter_context(tc.tile_pool(name="e4", bufs=2))
    apool = ctx.enter_context(tc.tile_pool(name="abuild", bufs=3))
    c1p = ctx.enter_context(tc.tile_pool(name="c1", bufs=2))
    perp = ctx.enter_context(tc.tile_pool(name="pers", bufs=1))
    lpool = ctx.enter_context(tc.tile_pool(name="lvin", bufs=4))
    ppool = ctx.enter_context(tc.tile_pool(name="prod", bufs=4))
    cpool = ctx.enter_context(tc.tile_pool(name="cout", bufs=4))
    spool = ctx.enter_context(tc.tile_pool(name="scal", bufs=8))

    hppool = ctx.enter_context(tc.tile_pool(name="hp", bufs=3, space="PSUM"))
    empool = ctx.enter_context(tc.tile_pool(name="emp", bufs=2, space="PSUM"))
    nppool = ctx.enter_context(tc.tile_pool(name="np", bufs=1, space="PSUM"))

    dram = ctx.enter_context(tc.tile_pool(name="dram", bufs=1, space="DRAM"))

    # ---- constants to SBUF (one DMA each, host pre-packed)
    w1_sb = const.tile([128, KS * MS * 128], BF16, tag="w1")
    nc.sync.dma_start(w1_sb[:, :], w1_d[:, :])
    w2_sb = const.tile([128, MS * L], BF16, tag="w2")
    nc.sync.dma_start(w2_sb[:, :], w2_d[:, :])
    b1_sb = const.tile([128, MS], F32, tag="b1")
    nc.sync.dma_start(b1_sb[:, :], b1_d[:, :])
    t9_sb = const.tile([128, L2], BF16, tag="t9")
    nc.sync.dma_start(t9_sb[:, :], t9_d[:, :])
    i81_sb = const.tile([1, L2], BF16, tag="i81")
    nc.sync.dma_start(i81_sb[:, :], i81_d[:, :])
    oh_sb = const.tile([64, NG * 72], F32, tag="oh")
    nc.sync.dma_start(oh_sb[:, :], oh_d[:, :])
    ones_sb = const.tile([64, 1], F32, tag="ones")
    nc.gpsimd.memset(ones_sb[:, :], 1.0)
    eps_sb = const.tile([128, 1], F32, tag="eps")
    nc.gpsimd.memset(eps_sb[:, :], EPS)

    acc_all = perp.tile([64, NG], F32, tag="accall")
    em0_all = perp.tile([1, NG * L], F32, tag="em0all")
    lm_all = perp.tile([128, 9], F32, tag="lmall")
    nc.gpsimd.memset(lm_all[:, :], 0.0)

    lv1 = dram.tile([T // 2, L2], BF16, tag="lv1")
    lv2 = dram.tile([T // 4, L2], BF16, tag="lv2")

    t9q = (t9_sb[0:64, :].rearrange("p (i j) -> p i j", i=L)
           .unsqueeze(1).broadcast_to([64, 2, L, L]))

    for _rep in range(reps):
        _emit_main(tc, nc, locals(), phases)

    ctx.close()


def _emit_main(tc, nc, env, phases=('load','stats','ln','tr','mm','em','tree')):
    g = None  # populated below from env
    (x_d, om_d, oe_d, on_d, lm_d, w1_sb, w2_sb, b1_sb, t9_sb, i81_sb, oh_sb,
     ones_sb, eps_sb, acc_all, em0_all, lm_all, lv1, lv2, t9q) = (
        env[k] for k in (
            "x_d", "om_d", "oe_d", "on_d", "lm_d", "w1_sb", "w2_sb", "b1_sb",
            "t9_sb", "i81_sb", "oh_sb", "ones_sb", "eps_sb", "acc_all",
            "em0_all", "lm_all", "lv1", "lv2", "t9q"))
    (xpool, jpool, stp, xnp, xntp, hpool, j3p, e4p, apool, c1p, lpool, ppool,
     cpool, spool, hppool, empool, nppool) = (
        env[k] for k in (
            "xpool", "jpool", "stp", "xnp", "xntp", "hpool", "j3p", "e4p",
            "apool", "c1p", "lpool", "ppool", "cpool", "spool", "hppool",
            "empool", "nppool"))

    # ================= per-group MLP + base matrices + L1 =================
    for g in range(NG):
        xnT = xntp.tile([128, KS * 512], BF16, tag="xnt")
        xnT_v = xnT[:, :].rearrange("p (k t) -> p k t", k=KS)
        for sp in range(2):           # two double-tiles of 2x128 tokens
            row = g * 512 + sp * 256
            x_t = xpool.tile([128, 2, H], BF16, tag="x")
            nc.sync.dma_start(
                x_t[:, :, :],
                x_d[row:row + 256, :].rearrange("(u p) h -> p u h", u=2))
            for u in range(2):
                s = sp * 2 + u
                xh = x_t[:, u, :]
                if 'stats' not in phases:
                    continue
                sx = stp.tile([128, 1], F32, tag="sx")
                nc.vector.reduce_sum(sx[:, :], xh, axis=AX.X)
                q = stp.tile([128, 1], F32, tag="q")
                junk2 = jpool.tile([128, H], BF16, tag="junk2")
                nc.vector.scalar_tensor_tensor(
                    out=junk2[:, :], in0=xh, scalar=1.0, in1=xh,
                    op0=AluOpType.mult, op1=AluOpType.mult,
                    accum_out=q[:, :])
                mean = stp.tile([128, 1], F32, tag="mean")
                nc.vector.tensor_scalar_mul(mean[:, :], sx[:, :], 1.0 / H)
                msq = stp.tile([128, 1], F32, tag="msq")
                nc.vector.tensor_tensor(msq[:, :], mean[:, :], mean[:, :],
                                        op=AluOpType.mult)
                var_t = stp.tile([128, 1], F32, tag="var")
                nc.vector.scalar_tensor_tensor(
                    out=var_t[:, :], in0=q[:, :], scalar=1.0 / H,
                    in1=msq[:, :],
                    op0=AluOpType.mult, op1=AluOpType.subtract)
                sd = stp.tile([128, 1], F32, tag="sd")
                nc.scalar.activation(sd[:, :], var_t[:, :], AF.Sqrt,
                                     bias=eps_sb[:, 0:1])
                rstd = stp.tile([128, 1], F32, tag="rstd")
                nc.vector.reciprocal(rstd[:, :], sd[:, :])
                nmr = stp.tile([128, 1], F32, tag="nmr")
                nc.vector.tensor_scalar(
                    nmr[:, :], mean[:, :], rstd[:, 0:1], -1.0,
                    op0=AluOpType.mult, op1=AluOpType.mult)
                if 'ln' not in phases:
                    continue
                xn_t = xnp.tile([128, H], BF16, tag="xn")
                nc.scalar.activation(xn_t[:, :], xh, AF.Identity,
                                     bias=nmr[:, 0:1], scale=rstd[:, 0:1])
                if 'tr' not in phases:
                    continue
                # one fat xbar transpose: [128 tok,1024 h]->[128 h',8 k,128 t]
                nc.sync.dma_start(
                    out=xnT_v[:, :, s * 128:(s + 1) * 128],
                    in_=xn_t[:, :], transpose=True)
        # ---- mm1 + gelu
        if 'mm' not in phases:
            continue
        h_sb = []
        for m in range(MS):
            hp = hppool.tile([128, 512], F32, tag="hp")
            for k in range(KS):
                nc.tensor.matmul(
                    hp[:, :],
                    w1_sb[:, (k * MS + m) * 128:(k * MS + m + 1) * 128],
                    xnT_v[:, k, :],
                    start=(k == 0), stop=(k == KS - 1))
            h_m = hpool.tile([128, 512], BF16, tag=f"h{m}")
            nc.scalar.activation(h_m[:, :], hp[:, :], AF.Gelu,
                                 bias=b1_sb[:, m:m + 1], scale=1.0)
            h_sb.append(h_m)
        # ---- mm2: emissions with token pairs on partitions: [64,(s,q,9)]
        em_p = empool.tile([64, 72], F32, tag="emp")
        for s in range(4):
            for qh in range(2):
                for m in range(MS):
                    nc.tensor.matmul(
                        em_p[:, s * 18 + qh * L: s * 18 + (qh + 1) * L],
                        h_sb[m][:, :].rearrange(
                            "p (s r two) -> p s two r", s=4, r=64, two=2
                        )[:, s, qh, :],
                        w2_sb[:, m * L:(m + 1) * L],
                        start=(m == 0), stop=(m == MS - 1))
        # numerator: sum_t em[t, tag_t]
        if 'em' not in phases:
            continue
        junk3 = j3p.tile([64, 72], F32, tag="junk3")
        nc.vector.scalar_tensor_tensor(
            out=junk3[:, :], in0=em_p[:, :], scalar=1.0,
            in1=oh_sb[:, g * 72:(g + 1) * 72],
            op0=AluOpType.mult, op1=AluOpType.mult,
            accum_out=acc_all[:, g:g + 1])
        # em0: token 0 = partition 0, s=0, q=0
        nc.vector.tensor_copy(em0_all[0:1, g * L:(g + 1) * L],
                              em_p[0:1, 0:L])
        # E = exp(em)
        E4 = e4p.tile([64, 72], BF16, tag="E4")
        nc.scalar.activation(E4[:, :], em_p[:, :], AF.Exp)
        # base matrices (pairs) + L1 combine in SBUF
        c1g = c1p.tile([64, 4 * L2], BF16, tag="c1g")
        for s in range(4):
            A_t = apool.tile([64, 2 * L2], BF16, tag="A")
            ev = (E4[:, s * 18:(s + 1) * 18]
                  .rearrange("p (two j) -> p two j", two=2)
                  .unsqueeze(2).broadcast_to([64, 2, L, L]))
            nc.vector.tensor_tensor(
                A_t[:, :].rearrange("p (two i j) -> p two i j", two=2, i=L),
                t9q, ev, op=AluOpType.mult)
            if s == 0:
                nc.vector.tensor_copy(A_t[0:1, 0:L2], i81_sb[0:1, :])
            # L1: pair product in SBUF
            P_t = ppool.tile([64, L2 * L], BF16, tag="prod1")
            _pairmul(nc, P_t[:, :].rearrange("p (i j k) -> p i j k",
                                             i=L, j=L), A_t[:, :], 64)
            with nc.allow_low_precision("fp32 ALU, single output round"):
                nc.vector.reduce_sum(
                    c1g[:, s * L2:(s + 1) * L2],
                    P_t[:, :].rearrange("p (f k) -> p f k", k=L),
                    axis=AX.X)
        # one store per group; lv1 row = g*256 + s*64 + r
        nc.sync.dma_start(
            lv1[:][g * 256:(g + 1) * 256, :].rearrange(
                "(s r) f -> r s f", s=4),
            c1g[:, :].rearrange("p (s f) -> p s f", s=4))

    # numerator partition-sum via ones-matmul
    np_p = nppool.tile([1, NG], F32, tag="npp")
    nc.tensor.matmul(np_p[:, :], ones_sb[:, :], acc_all[:, :])
    ne_sb = spool.tile([1, NG], F32, tag="ne")
    nc.vector.tensor_copy(ne_sb[:, :], np_p[:, :])
    nc.sync.dma_start(on_d[0:1, :], ne_sb[:, :])
    nc.sync.dma_start(oe_d[0:1, :], em0_all[0:1, :])

    # ================= CRF tree levels 2..9 =================
    if 'tree' not in phases:
        return
    # one packed load + one packed store per level
    srcs = {2: lv1, 3: lv2, 4: lv1, 5: lv2, 6: lv1, 7: lv2, 8: lv1, 9: lv2}
    for lvl in range(2, NLV + 1):
        rows_out = T >> lvl
        src = srcs[lvl]
        dst = srcs[lvl + 1] if lvl < NLV else None
        nt = (rows_out + 127) // 128          # tiles this level
        rows_t = min(128, rows_out)           # rows per tile
        # load whole level: in_t[p, t, 162] <- src rows 2*(128*t + p) ..+2
        in_t = lpool.tile([128, nt * 2 * L2], BF16, tag="lvin")
        srcv_ap = RAP(src[:].tensor, src[:].offset,
                      [[2 * L2, rows_t], [2 * 128 * L2, nt], [1, 2 * L2]])
        nc.sync.dma_start(
            in_t[:rows_t, :].rearrange("p (t f) -> p t f", t=nt), srcv_ap)
        outs_f32 = []
        c_pack = cpool.tile([128, nt * L2], BF16, tag="cpack")
        for ti in range(nt):
            P_t = ppool.tile([128, L2 * L], BF16, tag="prod")
            base = in_t[:rows_t, ti * 2 * L2:(ti + 1) * 2 * L2]
            _pairmul(nc, P_t[:rows_t, :].rearrange(
                "p (i j k) -> p i j k", i=L, j=L), base, rows_t)
            if lvl < RESCALE_FROM:
                with nc.allow_low_precision("fp32 ALU, single round"):
                    nc.vector.reduce_sum(
                        c_pack[:rows_t, ti * L2:(ti + 1) * L2],
                        P_t[:rows_t, :].rearrange("p (f k) -> p f k", k=L),
                        axis=AX.X)
                continue
            C_t = cpool.tile([128, L2], F32, tag="cout")
            nc.vector.reduce_sum(
                C_t[:rows_t, :],
                P_t[:rows_t, :].rearrange("p (f k) -> p f k", k=L),
                axis=AX.X)
            mx = spool.tile([128, 1], F32, tag="mx")
            nc.vector.reduce_max(mx[:rows_t, :], C_t[:rows_t, :], axis=AX.X)
            rmx = spool.tile([128, 1], F32, tag="rmx")
            nc.vector.reciprocal(rmx[:rows_t, :], mx[:rows_t, :])
            nc.vector.tensor_scalar_mul(C_t[:rows_t, :], C_t[:rows_t, :],
                                        rmx[:rows_t, 0:1])
            col, rr0 = LM_SLOTS[lvl][ti]
            nc.scalar.activation(lm_all[rr0:rr0 + rows_t, col:col + 1],
                                 mx[:rows_t, :], AF.Ln)
            if lvl < NLV:
                nc.vector.tensor_copy(
                    c_pack[:rows_t, ti * L2:(ti + 1) * L2], C_t[:rows_t, :])
            else:
                nc.sync.dma_start(om_d[:, :], C_t[:BPC, :])
        if lvl < NLV:
            dstv_ap = RAP(dst[:].tensor, dst[:].offset,
                          [[L2, rows_t], [128 * L2, nt], [1, L2]])
            nc.sync.dma_start(
                dstv_ap,
                c_pack[:rows_t, :].rearrange("p (t f) -> p t f", t=nt))

    nc.sync.dma_start(lm_d[:, :], lm_all[:, :])


def build_program(reps=1, phases=('load','stats','ln','tr','mm','em','tree')):
    nc = bass.Bass("TRN2", target_bir_lowering=False, debug=False)
    with tile.TileContext(nc) as tc:
        build_body(tc, reps=reps, phases=phases)
    _split_waits(nc)
    return nc


# ---------------------------------------------------------------------------
_CACHED = {}


def _get_program():
    if "nc" not in _CACHED:
        _CACHED["nc"] = build_program()
    return _CACHED["nc"]


def _host_prep(hidden_states, ln_gamma, ln_beta, W1, b1, W2, b2,
               start_trans, end_trans, trans, labels, attention_mask):
    x = np.ascontiguousarray(hidden_states, np.float32).reshape(B * S, H)
    tg = np.asarray(labels)
    W1p = (np.asarray(ln_gamma)[:, None] * np.asarray(W1)).astype(np.float32)
    b1p = (np.asarray(b1) + np.asarray(ln_beta) @ np.asarray(W1)).astype(
        np.float32)
    # w1 packed: [128, (k*MS+m)*128 + c] = W1p[k*128+p, m*128+c]
    w1t = np.ascontiguousarray(
        W1p.reshape(KS, 128, MS, 128).transpose(1, 0, 2, 3).reshape(
            128, KS * MS * 128)).astype(nbf16)
    # w2 packed: [128, m*9+l] = W2[m*128+p, l]
    w2t = np.ascontiguousarray(
        np.asarray(W2).reshape(MS, 128, L).transpose(1, 0, 2).reshape(
            128, MS * L)).astype(nbf16)
    T9b2 = np.exp(np.asarray(trans) + np.asarray(b2)[None, :])
    t9b = np.broadcast_to(T9b2.reshape(1, L2), (128, L2)).astype(nbf16)
    i81 = np.eye(L, dtype=np.float32).reshape(1, L2).astype(nbf16)
    b1_tile = np.ascontiguousarray(b1p.reshape(MS, 128).T, np.float32)

    oh_full = np.zeros((B * S, L), np.float32)
    oh_full[np.arange(B * S), tg.reshape(-1)] = 1.0

    num_table = (np.asarray(start_trans)[tg[:, 0]]
                 + np.asarray(trans)[tg[:, :-1], tg[:, 1:]].sum(1)
                 + np.asarray(end_trans)[tg[:, -1]]
                 + np.asarray(b2)[tg].sum(1)).astype(np.float64)

    xb = x.astype(nbf16)
    in_maps = []
    for c in range(NCORES):
        xc = np.ascontiguousarray(xb[c * T:(c + 1) * T])
        # oh layout: [r, g*72 + s*18 + q*9 + l] = onehot[g*512+s*128+2r+q, l]
        ohc = oh_full[c * T:(c + 1) * T].reshape(NG, 4, 64, 2, L)
        ohc = np.ascontiguousarray(
            ohc.transpose(2, 0, 1, 3, 4).reshape(64, NG * 72))
        in_maps.append({
            "x": xc, "w1": w1t, "w2": w2t, "b1": b1_tile,
            "t9": t9b, "i81": i81, "oh": ohc,
        })
    return in_maps, num_table


def _scale_from_lm(lm):
    """Sum per-example rescale logs from the packed lm_all [128, 8]."""
    sf = np.zeros(BPC, np.float64)
    for i in range(BPC):
        # L3: 512 rows, 64/example: col i//2, rows 64*(i%2)..+64
        sf[i] += lm[64 * (i % 2):64 * (i % 2) + 64, i // 2].sum()
        # L4: 256 rows, 32/example: col 4 + i//4, rows 32*(i%4)..+32
        sf[i] += lm[32 * (i % 4):32 * (i % 4) + 32, 4 + i // 4].sum()
        # L5: 128 rows, 16/example: col 6
        sf[i] += lm[16 * i:16 * i + 16, 6].sum()
        # L6: 64 rows, 8/example: col 7 rows 0..64
        sf[i] += lm[8 * i:8 * i + 8, 7].sum()
        # L7: 32 rows, 4/example: col 7 rows 64..96
        sf[i] += lm[64 + 4 * i:64 + 4 * i + 4, 7].sum()
        # L8: 16 rows, 2/example: col 7 rows 96..112
        sf[i] += lm[96 + 2 * i:96 + 2 * i + 2, 7].sum()
        # L9: 8 rows, 1/example: col 7 rows 112..120
        sf[i] += lm[i, 8]
    return sf


def _assemble(results, num_table, start_trans, end_trans, b2):
    start_trans = np.asarray(start_trans, np.float64)
    end_trans = np.asarray(end_trans, np.float64)
    b2 = np.asarray(b2, np.float64)
    llh = np.zeros(B, np.float64)
    for c in range(NCORES):
        r = results[c]
        Mf = np.asarray(r["out_m"], np.float64)
        Sf = _scale_from_lm(np.asarray(r["out_lm"], np.float64))
        em0 = np.asarray(r["out_em0"], np.float64).reshape(BPC, L)
        ne = np.asarray(r["out_ne"], np.float64)[0]
        logM = np.log(Mf).reshape(BPC, L, L) + Sf[:, None, None]
        score0 = start_trans[None, :] + em0 + b2[None, :]
        zz = score0[:, :, None] + logM + end_trans[None, None, :]
        mz = zz.max((1, 2), keepdims=True)
        denom = np.log(np.exp(zz - mz).sum((1, 2))) + mz[:, 0, 0]
        num = num_table[c * BPC:(c + 1) * BPC] + ne
        llh[c * BPC:(c + 1) * BPC] = num - denom
    return np.float32(-llh.mean())


def _reference_numpy(hidden_states, ln_gamma, ln_beta, W1, b1, W2, b2,
                     start_trans, end_trans, trans, labels, attention_mask):
    """Exact fallback (general mask/labels), pure numpy."""
    from scipy.special import erf
    x = np.asarray(hidden_states, np.float32)
    mu = x.mean(-1, keepdims=True)
    var = ((x - mu) ** 2).mean(-1, keepdims=True)
    xn = (x - mu) / np.sqrt(var + EPS) * ln_gamma + ln_beta
    hpre = xn @ W1 + b1
    h = 0.5 * hpre * (1 + erf(hpre / np.sqrt(2.0)))
    em = h @ W2 + b2
    labels = np.asarray(labels)
    mask = (labels != -100) & (np.asarray(attention_mask) == 1)
    mask[:, 0] = True
    tags = np.where(labels == -100, 0, labels)
    em_t = em.transpose(1, 0, 2).astype(np.float64)
    m = mask.T
    tg = tags.T
    mf = m.astype(np.float64)
    bar = np.arange(em_t.shape[1])
    em_sc = np.take_along_axis(em_t, tg[:, :, None], 2)[:, :, 0]
    pair = np.asarray(trans)[tg[:-1], tg[1:]]
    num = (np.asarray(start_trans)[tg[0]] + em_sc[0]
           + ((pair + em_sc[1:]) * mf[1:]).sum(0))
    seq_ends = m.astype(np.int64).sum(0) - 1
    num = num + np.asarray(end_trans)[tg[seq_ends, bar], ]
    score = np.asarray(start_trans)[None, :] + em_t[0]
    for i in range(1, em_t.shape[0]):
        z = score[:, :, None] + np.asarray(trans)[None] + em_t[i][:, None, :]
        zm = z.max(1, keepdims=True)
        nxt = np.log(np.exp(z - zm).sum(1)) + zm[:, 0, :]
        score = np.where(m[i][:, None], nxt, score)
    z = score + np.asarray(end_trans)[None, :]
    zm = z.max(1, keepdims=True)
    denom = np.log(np.exp(z - zm).sum(1)) + zm[:, 0]
    return np.float32(-(num - denom).mean())


def kernel(**inputs):
    labels = np.asarray(inputs["labels"])
    am = np.asarray(inputs["attention_mask"])
    if not ((am == 1).all() and (labels >= 0).all() and (labels < L).all()):
        return _reference_numpy(**inputs)

    from concourse.bass_utils import run_bass_kernel_spmd
    nc = _get_program()
    in_maps, num_table = _host_prep(**inputs)
    res = run_bass_kernel_spmd(nc, in_maps, list(range(NCORES)))
    return _assemble(res.results, num_table,
                     inputs["start_trans"], inputs["end_trans"], inputs["b2"])



# revision 2
# speedup vs baseline: 404.9536x; 404.9536x over previous
"""Trainium2 Bass kernel for DebertaV3+CRF token-classification loss.

Computes: LayerNorm -> Linear(1024,512) -> GELU(exact) -> Linear(512,9)
-> CRF negative log-likelihood (mean over batch).

Strategy: data-parallel over batch across 8 NeuronCores (8 examples each).
The CRF partition function is computed as a balanced tree of 9x9 matrix
products in probability domain (log-rescaled from level 3 on), which turns
the 511-step sequential scan into ~9 parallel levels of elementwise
multiply+reduce on the vector engine. mm2 emits emissions with consecutive
token PAIRS on one partition so tree level 1 runs directly in SBUF.

Self-contained: only imports numpy/ml_dtypes and the system concourse repo.
"""

import sys

for _p in ("/opt/trn_rl_repo", "/root/.axon_site/_ro/trn_rl_repo"):
    if _p not in sys.path:
        sys.path.append(_p)

import numpy as np
import ml_dtypes

import concourse.bass as bass
import concourse.tile as tile
import concourse.mybir as mybir
from concourse.alu_op_type import AluOpType
from bass_rust import AP as RAP, ScopedClock

BF16 = mybir.dt.bfloat16
F32 = mybir.dt.float32
AX = mybir.AxisListType
AF = mybir.ActivationFunctionType
nbf16 = ml_dtypes.bfloat16

B, S, H, L = 64, 512, 1024, 9
EPS = 1e-5
NCORES = 8
BPC = B // NCORES          # examples per core
T = BPC * S                # tokens per core (4096)
NG = BPC                   # token groups of 512 = one example each
KS = H // 128              # 8 k-slices
MS = 512 // 128            # 4 ch-slices
L2 = L * L                 # 81
RESCALE_FROM = 3           # rescale tree levels >= this
NLV = 9                    # tree levels (512 tokens/example)

# lm_all packing: level -> list of (col, row0) per tile
LM_SLOTS = {
    3: [(0, 0), (1, 0), (2, 0), (3, 0)],
    4: [(4, 0), (5, 0)],
    5: [(6, 0)],
    6: [(7, 0)],
    7: [(7, 64)],
    8: [(7, 96)],
    9: [(8, 0)],
}


# ---------------------------------------------------------------------------
# TileContext drain patch: this walrus build rejects >1 semaphore wait on the
# final SP drain ("Too many sync wait commands"); split waits across nops.
def _patched_drain_and_barrier(self, tick_clock, wait_clock):
    drain_inst = self.nc.sync.drain()
    wait_clock.add_sem_waits(
        drain_inst.ins, ScopedClock({None: tick_clock.global_clock})
    )
    si = drain_inst.ins.sync_info
    waits = list(si.on_wait) if si and si.on_wait else []
    if len(waits) > 1:
        si.on_wait = []
        insts = self.nc.cur_bb.bb.instructions
        assert insts[-1] is drain_inst.ins
        insts.pop()
        for w in waits:
            nop = self.nc.sync.nop(nofuse=True)
            nsi = nop.ins.sync_info
            if nsi is None:
                nop.ins.sync_info = mybir.SyncInfo(on_wait=[w], on_update=[])
            else:
                nsi.on_wait = [w]
        insts.append(drain_inst.ins)
    self.nc.all_engine_barrier()
    assert self.sems is not None
    popped = self.nc._tile_sem_poison_stack.pop()
    assert popped is self._sem_poison
    self.nc.clear_and_free_semaphores(list(self.sems.allocated().values()))
    self.nc.all_engine_barrier()


tile.TileContext._drain_and_barrier = _patched_drain_and_barrier


def _split_waits(nc, maxw=1):
    """This walrus build rejects instructions with more than ~1-2 semaphore
    waits; hoist extras onto same-engine nops inserted before the instruction."""
    for f in nc.m.functions:
        for bb in f.blocks:
            insts = bb.instructions
            new = []
            changed = False
            for inst in list(insts):
                si = inst.sync_info
                waits = list(si.on_wait) if si and si.on_wait else []
                if len(waits) > maxw:
                    changed = True
                    si.on_wait = waits[-maxw:]
                    for w in waits[:-maxw]:
                        nop = nc.engines[inst.engine].nop(nofuse=True)
                        cb = nc.cur_bb.bb.instructions
                        assert cb[-1] is nop.ins
                        cb.pop()
                        if nop.ins.sync_info is None:
                            nop.ins.sync_info = mybir.SyncInfo(
                                on_wait=[w], on_update=[])
                        else:
                            nop.ins.sync_info.on_wait = [w]
                        new.append(nop.ins)
                new.append(inst)
            if changed:
                while len(insts):
                    insts.pop()
                for i in new:
                    insts.append(i)


def _pairmul(nc, out_ap, base, rows, engine=None):
    """out[p,i,j] = sum_k A[p,i,k]*B[p,k,j] first half: the multiply.
    base: AP of a [rows, 162] tile region (A | B). Returns nothing."""
    p_ent = list(base.ap[0])
    a_ap = RAP(base.tensor, base.offset, [p_ent, [L, L], [0, L], [1, L]])
    b_ap = RAP(base.tensor, base.offset + L2, [p_ent, [0, L], [1, L], [L, L]])
    eng = engine or nc.vector
    eng.tensor_tensor(out_ap, a_ap, b_ap, op=AluOpType.mult)


# ---------------------------------------------------------------------------
def build_body(tc, reps=1, phases=('load','stats','ln','tr','mm','em','tree')):
    nc = tc.nc
    x_d = nc.dram_tensor("x", [T, H], BF16, kind="ExternalInput").ap()
    w1_d = nc.dram_tensor("w1", [128, KS * MS * 128], BF16,
                          kind="ExternalInput").ap()
    w2_d = nc.dram_tensor("w2", [128, MS * L], BF16, kind="ExternalInput").ap()
    b1_d = nc.dram_tensor("b1", [128, MS], F32, kind="ExternalInput").ap()
    t9_d = nc.dram_tensor("t9", [128, L2], BF16, kind="ExternalInput").ap()
    i81_d = nc.dram_tensor("i81", [1, L2], BF16, kind="ExternalInput").ap()
    oh_d = nc.dram_tensor("oh", [64, NG * 72], F32, kind="ExternalInput").ap()

    om_d = nc.dram_tensor("out_m", [BPC, L2], F32, kind="ExternalOutput").ap()
    oe_d = nc.dram_tensor("out_em0", [1, NG * L], F32,
                          kind="ExternalOutput").ap()
    on_d = nc.dram_tensor("out_ne", [1, NG], F32, kind="ExternalOutput").ap()
    lm_d = nc.dram_tensor("out_lm", [128, 9], F32, kind="ExternalOutput").ap()

    from contextlib import ExitStack
    ctx = tc._build_ctx = ExitStack()
    ctx.__enter__()

    const = ctx.enter_context(tc.tile_pool(name="const", bufs=1))
    xpool = ctx.enter_context(tc.tile_pool(name="xp", bufs=3))
    jpool = ctx.enter_context(tc.tile_pool(name="junk", bufs=2))
    stp = ctx.enter_context(tc.tile_pool(name="stats", bufs=8))
    xnp = ctx.enter_context(tc.tile_pool(name="xn", bufs=3))
    xntp = ctx.enter_context(tc.tile_pool(name="xnt", bufs=2))
    hpool = ctx.enter_context(tc.tile_pool(name="h", bufs=2))
    j3p = ctx.enter_context(tc.tile_pool(name="junk3", bufs=2))
    e4p = ctx.enter_context(tc.tile_pool(name="e4", bufs=2))
    apool = ctx.enter_context(tc.tile_pool(name="abuild", bufs=3))
    c1p = ctx.enter_context(tc.tile_pool(name="c1", bufs=2))
    perp = ctx.enter_context(tc.tile_pool(name="pers", bufs=1))
    lpool = ctx.enter_context(tc.tile_pool(name="lvin", bufs=4))
    ppool = ctx.enter_context(tc.tile_pool(name="prod", bufs=4))
    cpool = ctx.enter_context(tc.tile_pool(name="cout", bufs=4))
    spool = ctx.enter_context(tc.tile_pool(name="scal", bufs=8))

    hppool = ctx.enter_context(tc.tile_pool(name="hp", bufs=3, space="PSUM"))
    empool = ctx.enter_context(tc.tile_pool(name="emp", bufs=2, space="PSUM"))
    nppool = ctx.enter_context(tc.tile_pool(name="np", bufs=1, space="PSUM"))

    dram = ctx.enter_context(tc.tile_pool(name="dram", bufs=1, space="DRAM"))

    # ---- constants to SBUF (one DMA each, host pre-packed)
    w1_sb = const.tile([128, KS * MS * 128], BF16, tag="w1")
    nc.sync.dma_start(w1_sb[:, :], w1_d[:, :])
    w2_sb = const.tile([128, MS * L], BF16, tag="w2")
    nc.sync.dma_start(w2_sb[:, :], w2_d[:, :])
    b1_sb = const.tile([128, MS], F32, tag="b1")
    nc.sync.dma_start(b1_sb[:, :], b1_d[:, :])
    t9_sb = const.tile([128, L2], BF16, tag="t9")
    nc.sync.dma_start(t9_sb[:, :], t9_d[:, :])
    i81_sb = const.tile([1, L2], BF16, tag="i81")
    nc.sync.dma_start(i81_sb[:, :], i81_d[:, :])
    oh_sb = const.tile([64, NG * 72], F32, tag="oh")
    nc.sync.dma_start(oh_sb[:, :], oh_d[:, :])
    ones_sb = const.tile([64, 1], F32, tag="ones")
    nc.gpsimd.memset(ones_sb[:, :], 1.0)
    eps_sb = const.tile([128, 1], F32, tag="eps")
    nc.gpsimd.memset(eps_sb[:, :], EPS)

    acc_all = perp.tile([64, NG], F32, tag="accall")
    em0_all = perp.tile([1, NG * L], F32, tag="em0all")
    lm_all = perp.tile([128, 9], F32, tag="lmall")
    nc.gpsimd.memset(lm_all[:, :], 0.0)

    lv1 = dram.tile([T // 2, L2], BF16, tag="lv1")
    lv2 = dram.tile([T // 4, L2], BF16, tag="lv2")

    t9q = (t9_sb[0:64, :].rearrange("p (i j) -> p i j", i=L)
           .unsqueeze(1).broadcast_to([64, 2, L, L]))

    for _rep in range(reps):
        _emit_main(tc, nc, locals(), phases)

    ctx.close()


def _emit_main(tc, nc, env, phases=('load','stats','ln','tr','mm','em','tree')):
    g = None  # populated below from env
    (x_d, om_d, oe_d, on_d, lm_d, w1_sb, w2_sb, b1_sb, t9_sb, i81_sb, oh_sb,
     ones_sb, eps_sb, acc_all, em0_all, lm_all, lv1, lv2, t9q) = (
        env[k] for k in (
            "x_d", "om_d", "oe_d", "on_d", "lm_d", "w1_sb", "w2_sb", "b1_sb",
            "t9_sb", "i81_sb", "oh_sb", "ones_sb", "eps_sb", "acc_all",
            "em0_all", "lm_all", "lv1", "lv2", "t9q"))
    (xpool, jpool, stp, xnp, xntp, hpool, j3p, e4p, apool, c1p, lpool, ppool,
     cpool, spool, hppool, empool, nppool) = (
        env[k] for k in (
            "xpool", "jpool", "stp", "xnp", "xntp", "hpool", "j3p", "e4p",
            "apool", "c1p", "lpool", "ppool", "cpool", "spool", "hppool",
            "empool", "nppool"))

    # ================= per-group MLP + base matrices + L1 =================
    for g in range(NG):
        xnT = xntp.tile([128, KS * 512], BF16, tag="xnt")
        xnT_v = xnT[:, :].rearrange("p (k t) -> p k t", k=KS)
        for sp in range(2):           # two double-tiles of 2x128 tokens
            row = g * 512 + sp * 256
            x_t = xpool.tile([128, 2, H], BF16, tag="x")
            nc.sync.dma_start(
                x_t[:, :, :],
                x_d[row:row + 256, :].rearrange("(u p) h -> p u h", u=2))
            for u in range(2):
                s = sp * 2 + u
                xh = x_t[:, u, :]
                if 'stats' not in phases:
                    continue
                sx = stp.tile([128, 1], F32, tag="sx")
                nc.vector.reduce_sum(sx[:, :], xh, axis=AX.X)
                q = stp.tile([128, 1], F32, tag="q")
                junk2 = jpool.tile([128, H], BF16, tag="junk2")
                nc.vector.scalar_tensor_tensor(
                    out=junk2[:, :], in0=xh, scalar=1.0, in1=xh,
                    op0=AluOpType.mult, op1=AluOpType.mult,
                    accum_out=q[:, :])
                mean = stp.tile([128, 1], F32, tag="mean")
                nc.vector.tensor_scalar_mul(mean[:, :], sx[:, :], 1.0 / H)
                msq = stp.tile([128, 1], F32, tag="msq")
                nc.vector.tensor_tensor(msq[:, :], mean[:, :], mean[:, :],
                                        op=AluOpType.mult)
                var_t = stp.tile([128, 1], F32, tag="var")
                nc.vector.scalar_tensor_tensor(
                    out=var_t[:, :], in0=q[:, :], scalar=1.0 / H,
                    in1=msq[:, :],
                    op0=AluOpType.mult, op1=AluOpType.subtract)
                sd = stp.tile([128, 1], F32, tag="sd")
                nc.scalar.activation(sd[:, :], var_t[:, :], AF.Sqrt,
                                     bias=eps_sb[:, 0:1])
                rstd = stp.tile([128, 1], F32, tag="rstd")
                nc.vector.reciprocal(rstd[:, :], sd[:, :])
                nmr = stp.tile([128, 1], F32, tag="nmr")
                nc.vector.tensor_scalar(
                    nmr[:, :], mean[:, :], rstd[:, 0:1], -1.0,
                    op0=AluOpType.mult, op1=AluOpType.mult)
                if 'ln' not in phases:
                    continue
                xn_t = xnp.tile([128, H], BF16, tag="xn")
                nc.scalar.activation(xn_t[:, :], xh, AF.Identity,
                                     bias=nmr[:, 0:1], scale=rstd[:, 0:1])
                if 'tr' not in phases:
                    continue
                # one fat xbar transpose: [128 tok,1024 h]->[128 h',8 k,128 t]
                nc.sync.dma_start(
                    out=xnT_v[:, :, s * 128:(s + 1) * 128],
                    in_=xn_t[:, :], transpose=True)
        # ---- mm1 + gelu
        if 'mm' not in phases:
            continue
        h_sb = []
        for m in range(MS):
            hp = hppool.tile([128, 512], F32, tag="hp")
            for k in range(KS):
                nc.tensor.matmul(
                    hp[:, :],
                    w1_sb[:, (k * MS + m) * 128:(k * MS + m + 1) * 128],
                    xnT_v[:, k, :],
                    start=(k == 0), stop=(k == KS - 1))
            h_m = hpool.tile([128, 512], BF16, tag=f"h{m}")
            nc.scalar.activation(h_m[:, :], hp[:, :], AF.Gelu,
                                 bias=b1_sb[:, m:m + 1], scale=1.0)
            h_sb.append(h_m)
        # ---- mm2: emissions with token pairs on partitions: [64,(s,q,9)]
        em_p = empool.tile([64, 72], F32, tag="emp")
        for s in range(4):
            for qh in range(2):
                for m in range(MS):
                    nc.tensor.matmul(
                        em_p[:, s * 18 + qh * L: s * 18 + (qh + 1) * L],
                        h_sb[m][:, :].rearrange(
                            "p (s r two) -> p s two r", s=4, r=64, two=2
                        )[:, s, qh, :],
                        w2_sb[:, m * L:(m + 1) * L],
                        start=(m == 0), stop=(m == MS - 1))
        # numerator: sum_t em[t, tag_t]
        if 'em' not in phases:
            continue
        junk3 = j3p.tile([64, 72], F32, tag="junk3")
        nc.vector.scalar_tensor_tensor(
            out=junk3[:, :], in0=em_p[:, :], scalar=1.0,
            in1=oh_sb[:, g * 72:(g + 1) * 72],
            op0=AluOpType.mult, op1=AluOpType.mult,
            accum_out=acc_all[:, g:g + 1])
        # em0: token 0 = partition 0, s=0, q=0
        nc.vector.tensor_copy(em0_all[0:1, g * L:(g + 1) * L],
                              em_p[0:1, 0:L])
        # E = exp(em)
        E4 = e4p.tile([64, 72], BF16, tag="E4")
        nc.scalar.activation(E4[:, :], em_p[:, :], AF.Exp)
        # base matrices (pairs) + L1 combine in SBUF
        c1g = c1p.tile([64, 4 * L2], BF16, tag="c1g")
        for s in range(4):
            A_t = apool.tile([64, 2 * L2], BF16, tag="A")
            ev = (E4[:, s * 18:(s + 1) * 18]
                  .rearrange("p (two j) -> p two j", two=2)
                  .unsqueeze(2).broadcast_to([64, 2, L, L]))
            nc.vector.tensor_tensor(
                A_t[:, :].rearrange("p (two i j) -> p two i j", two=2, i=L),
                t9q, ev, op=AluOpType.mult)
            if s == 0:
                nc.vector.tensor_copy(A_t[0:1, 0:L2], i81_sb[0:1, :])
            # L1: pair product in SBUF
            P_t = ppool.tile([64, L2 * L], BF16, tag="prod1")
            _pairmul(nc, P_t[:, :].rearrange("p (i j k) -> p i j k",
                                             i=L, j=L), A_t[:, :], 64)
            with nc.allow_low_precision("fp32 ALU, single output round"):
                nc.vector.reduce_sum(
                    c1g[:, s * L2:(s + 1) * L2],
                    P_t[:, :].rearrange("p (f k) -> p f k", k=L),
                    axis=AX.X)
        # one store per group; lv1 row = g*256 + s*64 + r
        nc.sync.dma_start(
            lv1[:][g * 256:(g + 1) * 256, :].rearrange(
                "(s r) f -> r s f", s=4),
            c1g[:, :].rearrange("p (s f) -> p s f", s=4))

    # numerator partition-sum via ones-matmul
    if 'em' in phases:
        np_p = nppool.tile([1, NG], F32, tag="npp")
        nc.tensor.matmul(np_p[:, :], ones_sb[:, :], acc_all[:, :])
        ne_sb = spool.tile([1, NG], F32, tag="ne")
        nc.vector.tensor_copy(ne_sb[:, :], np_p[:, :])
        nc.sync.dma_start(on_d[0:1, :], ne_sb[:, :])
        nc.sync.dma_start(oe_d[0:1, :], em0_all[0:1, :])

    # ================= CRF tree levels 2..9 =================
    if 'tree' not in phases:
        return
    # one packed load + one packed store per level
    srcs = {2: lv1, 3: lv2, 4: lv1, 5: lv2, 6: lv1, 7: lv2, 8: lv1, 9: lv2}
    for lvl in range(2, NLV + 1):
        rows_out = T >> lvl
        src = srcs[lvl]
        dst = srcs[lvl + 1] if lvl < NLV else None
        nt = (rows_out + 127) // 128          # tiles this level
        rows_t = min(128, rows_out)           # rows per tile
        # load whole level: in_t[p, t, 162] <- src rows 2*(128*t + p) ..+2
        in_t = lpool.tile([128, nt * 2 * L2], BF16, tag="lvin")
        srcv_ap = RAP(src[:].tensor, src[:].offset,
                      [[2 * L2, rows_t], [2 * 128 * L2, nt], [1, 2 * L2]])
        nc.sync.dma_start(
            in_t[:rows_t, :].rearrange("p (t f) -> p t f", t=nt), srcv_ap)
        outs_f32 = []
        c_pack = cpool.tile([128, nt * L2], BF16, tag="cpack")
        for ti in range(nt):
            P_t = ppool.tile([128, L2 * L], BF16, tag="prod")
            base = in_t[:rows_t, ti * 2 * L2:(ti + 1) * 2 * L2]
            _pairmul(nc, P_t[:rows_t, :].rearrange(
                "p (i j k) -> p i j k", i=L, j=L), base, rows_t)
            if lvl < RESCALE_FROM:
                with nc.allow_low_precision("fp32 ALU, single round"):
                    nc.vector.reduce_sum(
                        c_pack[:rows_t, ti * L2:(ti + 1) * L2],
                        P_t[:rows_t, :].rearrange("p (f k) -> p f k", k=L),
                        axis=AX.X)
                continue
            C_t = cpool.tile([128, L2], F32, tag="cout")
            nc.vector.reduce_sum(
                C_t[:rows_t, :],
                P_t[:rows_t, :].rearrange("p (f k) -> p f k", k=L),
                axis=AX.X)
            mx = spool.tile([128, 1], F32, tag="mx")
            nc.vector.reduce_max(mx[:rows_t, :], C_t[:rows_t, :], axis=AX.X)
            rmx = spool.tile([128, 1], F32, tag="rmx")
            nc.vector.reciprocal(rmx[:rows_t, :], mx[:rows_t, :])
            nc.vector.tensor_scalar_mul(C_t[:rows_t, :], C_t[:rows_t, :],
                                        rmx[:rows_t, 0:1])
            col, rr0 = LM_SLOTS[lvl][ti]
            nc.scalar.activation(lm_all[rr0:rr0 + rows_t, col:col + 1],
                                 mx[:rows_t, :], AF.Ln)
            if lvl < NLV:
                nc.vector.tensor_copy(
                    c_pack[:rows_t, ti * L2:(ti + 1) * L2], C_t[:rows_t, :])
            else:
                nc.sync.dma_start(om_d[:, :], C_t[:BPC, :])
        if lvl < NLV:
            dstv_ap = RAP(dst[:].tensor, dst[:].offset,
                          [[L2, rows_t], [128 * L2, nt], [1, L2]])
            nc.sync.dma_start(
                dstv_ap,
                c_pack[:rows_t, :].rearrange("p (t f) -> p t f", t=nt))

    nc.sync.dma_start(lm_d[:, :], lm_all[:, :])


def build_program(reps=1, phases=('load','stats','ln','tr','mm','em','tree')):
    nc = bass.Bass("TRN2", target_bir_lowering=False, debug=False)
    with tile.TileContext(nc) as tc:
        build_body(tc, reps=reps, phases=phases)
    _split_waits(nc)
    return nc


# ---------------------------------------------------------------------------
_CACHED = {}


def _get_program():
    if "nc" not in _CACHED:
        _CACHED["nc"] = build_program()
    return _CACHED["nc"]


def _host_prep(hidden_states, ln_gamma, ln_beta, W1, b1, W2, b2,
               start_trans, end_trans, trans, labels, attention_mask):
    x = np.ascontiguousarray(hidden_states, np.float32).reshape(B * S, H)
    tg = np.asarray(labels)
    W1p = (np.asarray(ln_gamma)[:, None] * np.asarray(W1)).astype(np.float32)
    b1p = (np.asarray(b1) + np.asarray(ln_beta) @ np.asarray(W1)).astype(
        np.float32)
    # w1 packed: [128, (k*MS+m)*128 + c] = W1p[k*128+p, m*128+c]
    w1t = np.ascontiguousarray(
        W1p.reshape(KS, 128, MS, 128).transpose(1, 0, 2, 3).reshape(
            128, KS * MS * 128)).astype(nbf16)
    # w2 packed: [128, m*9+l] = W2[m*128+p, l]
    w2t = np.ascontiguousarray(
        np.asarray(W2).reshape(MS, 128, L).transpose(1, 0, 2).reshape(
            128, MS * L)).astype(nbf16)
    T9b2 = np.exp(np.asarray(trans) + np.asarray(b2)[None, :])
    t9b = np.broadcast_to(T9b2.reshape(1, L2), (128, L2)).astype(nbf16)
    i81 = np.eye(L, dtype=np.float32).reshape(1, L2).astype(nbf16)
    b1_tile = np.ascontiguousarray(b1p.reshape(MS, 128).T, np.float32)

    oh_full = np.zeros((B * S, L), np.float32)
    oh_full[np.arange(B * S), tg.reshape(-1)] = 1.0

    num_table = (np.asarray(start_trans)[tg[:, 0]]
                 + np.asarray(trans)[tg[:, :-1], tg[:, 1:]].sum(1)
                 + np.asarray(end_trans)[tg[:, -1]]
                 + np.asarray(b2)[tg].sum(1)).astype(np.float64)

    xb = x.astype(nbf16)
    in_maps = []
    for c in range(NCORES):
        xc = np.ascontiguousarray(xb[c * T:(c + 1) * T])
        # oh layout: [r, g*72 + s*18 + q*9 + l] = onehot[g*512+s*128+2r+q, l]
        ohc = oh_full[c * T:(c + 1) * T].reshape(NG, 4, 64, 2, L)
        ohc = np.ascontiguousarray(
            ohc.transpose(2, 0, 1, 3, 4).reshape(64, NG * 72))
        in_maps.append({
            "x": xc, "w1": w1t, "w2": w2t, "b1": b1_tile,
            "t9": t9b, "i81": i81, "oh": ohc,
        })
    return in_maps, num_table


def _scale_from_lm(lm):
    """Sum per-example rescale logs from the packed lm_all [128, 8]."""
    sf = np.zeros(BPC, np.float64)
    for i in range(BPC):
        # L3: 512 rows, 64/example: col i//2, rows 64*(i%2)..+64
        sf[i] += lm[64 * (i % 2):64 * (i % 2) + 64, i // 2].sum()
        # L4: 256 rows, 32/example: col 4 + i//4, rows 32*(i%4)..+32
        sf[i] += lm[32 * (i % 4):32 * (i % 4) + 32, 4 + i // 4].sum()
        # L5: 128 rows, 16/example: col 6
        sf[i] += lm[16 * i:16 * i + 16, 6].sum()
        # L6: 64 rows, 8/example: col 7 rows 0..64
        sf[i] += lm[8 * i:8 * i + 8, 7].sum()
        # L7: 32 rows, 4/example: col 7 rows 64..96
        sf[i] += lm[64 + 4 * i:64 + 4 * i + 4, 7].sum()
        # L8: 16 rows, 2/example: col 7 rows 96..112
        sf[i] += lm[96 + 2 * i:96 + 2 * i + 2, 7].sum()
        # L9: 8 rows, 1/example: col 7 rows 112..120
        sf[i] += lm[i, 8]
    return sf


def _assemble(results, num_table, start_trans, end_trans, b2):
    start_trans = np.asarray(start_trans, np.float64)
    end_trans = np.asarray(end_trans, np.float64)
    b2 = np.asarray(b2, np.float64)
    llh = np.zeros(B, np.float64)
    for c in range(NCORES):
        r = results[c]
        Mf = np.asarray(r["out_m"], np.float64)
        Sf = _scale_from_lm(np.asarray(r["out_lm"], np.float64))
        em0 = np.asarray(r["out_em0"], np.float64).reshape(BPC, L)
        ne = np.asarray(r["out_ne"], np.float64)[0]
        logM = np.log(Mf).reshape(BPC, L, L) + Sf[:, None, None]
        score0 = start_trans[None, :] + em0 + b2[None, :]
        zz = score0[:, :, None] + logM + end_trans[None, None, :]
        mz = zz.max((1, 2), keepdims=True)
        denom = np.log(np.exp(zz - mz).sum((1, 2))) + mz[:, 0, 0]
        num = num_table[c * BPC:(c + 1) * BPC] + ne
        llh[c * BPC:(c + 1) * BPC] = num - denom
    return np.float32(-llh.mean())


def _reference_numpy(hidden_states, ln_gamma, ln_beta, W1, b1, W2, b2,
                     start_trans, end_trans, trans, labels, attention_mask):
    """Exact fallback (general mask/labels), pure numpy."""
    from scipy.special import erf
    x = np.asarray(hidden_states, np.float32)
    mu = x.mean(-1, keepdims=True)
    var = ((x - mu) ** 2).mean(-1, keepdims=True)
    xn = (x - mu) / np.sqrt(var + EPS) * ln_gamma + ln_beta
    hpre = xn @ W1 + b1
    h = 0.5 * hpre * (1 + erf(hpre / np.sqrt(2.0)))
    em = h @ W2 + b2
    labels = np.asarray(labels)
    mask = (labels != -100) & (np.asarray(attention_mask) == 1)
    mask[:, 0] = True
    tags = np.where(labels == -100, 0, labels)
    em_t = em.transpose(1, 0, 2).astype(np.float64)
    m = mask.T
    tg = tags.T
    mf = m.astype(np.float64)
    bar = np.arange(em_t.shape[1])
    em_sc = np.take_along_axis(em_t, tg[:, :, None], 2)[:, :, 0]
    pair = np.asarray(trans)[tg[:-1], tg[1:]]
    num = (np.asarray(start_trans)[tg[0]] + em_sc[0]
           + ((pair + em_sc[1:]) * mf[1:]).sum(0))
    seq_ends = m.astype(np.int64).sum(0) - 1
    num = num + np.asarray(end_trans)[tg[seq_ends, bar], ]
    score = np.asarray(start_trans)[None, :] + em_t[0]
    for i in range(1, em_t.shape[0]):
        z = score[:, :, None] + np.asarray(trans)[None] + em_t[i][:, None, :]
        zm = z.max(1, keepdims=True)
        nxt = np.log(np.exp(z - zm).sum(1)) + zm[:, 0, :]
        score = np.where(m[i][:, None], nxt, score)
    z = score + np.asarray(end_trans)[None, :]
    zm = z.max(1, keepdims=True)
    denom = np.log(np.exp(z - zm).sum(1)) + zm[:, 0]
    return np.float32(-(num - denom).mean())


def kernel(**inputs):
    labels = np.asarray(inputs["labels"])
    am = np.asarray(inputs["attention_mask"])
    if not ((am == 1).all() and (labels >= 0).all() and (labels < L).all()):
        return _reference_numpy(**inputs)

    from concourse.bass_utils import run_bass_kernel_spmd
    nc = _get_program()
    in_maps, num_table = _host_prep(**inputs)
    res = run_bass_kernel_spmd(nc, in_maps, list(range(NCORES)))
    return _assemble(res.results, num_table,
                     inputs["start_trans"], inputs["end_trans"], inputs["b2"])



# revision 23
# speedup vs baseline: 556.4616x; 1.3741x over previous
"""Trainium2 Bass kernel for DebertaV3+CRF token-classification loss.

Computes: LayerNorm -> Linear(1024,512) -> GELU(exact) -> Linear(512,9)
-> CRF negative log-likelihood (mean over batch).

Strategy: data-parallel over batch across 8 NeuronCores (8 examples each).
The CRF partition function is computed as a balanced tree of 9x9 matrix
products in probability domain (log-rescaled from level 3 on), which turns
the 511-step sequential scan into ~9 parallel levels of elementwise
multiply+reduce on the vector engine. mm2 emits emissions with consecutive
token PAIRS on one partition so tree level 1 runs directly in SBUF.

Self-contained: only imports numpy/ml_dtypes and the system concourse repo.
"""

import sys

for _p in ("/opt/trn_rl_repo", "/root/.axon_site/_ro/trn_rl_repo"):
    if _p not in sys.path:
        sys.path.append(_p)

import numpy as np
import ml_dtypes

import concourse.bass as bass
import concourse.tile as tile
import concourse.mybir as mybir
from concourse.alu_op_type import AluOpType
from bass_rust import AP as RAP, ScopedClock

BF16 = mybir.dt.bfloat16
F32 = mybir.dt.float32
AX = mybir.AxisListType
AF = mybir.ActivationFunctionType
nbf16 = ml_dtypes.bfloat16

B, S, H, L = 64, 512, 1024, 9
EPS = 1e-5
NCORES = 8
BPC = B // NCORES          # examples per core
T = BPC * S                # tokens per core (4096)
NG = BPC                   # token groups of 512 = one example each
KS = H // 128              # 8 k-slices
MS = 512 // 128            # 4 ch-slices
L2 = L * L                 # 81
NLV = 9                    # tree levels (512 tokens/example)
# Rescale only at these levels: unrescaled growth peaks at ~e^39 by L4,
# then <=e^23 by L7 and <=e^30 by L9 (bf16/f32 max ~e^88) -- safe.
RESCALE_LEVELS = (4, 7, 9)

# lm_all packing: level -> list of (col, row0) per tile
LM_SLOTS = {
    4: [(0, 0), (1, 0)],
    7: [(2, 0)],
    9: [(3, 0)],
}


# ---------------------------------------------------------------------------
# TileContext drain patch: this walrus build rejects >1 semaphore wait on the
# final SP drain ("Too many sync wait commands"); split waits across nops.
def _patched_drain_and_barrier(self, tick_clock, wait_clock):
    drain_inst = self.nc.sync.drain()
    wait_clock.add_sem_waits(
        drain_inst.ins, ScopedClock({None: tick_clock.global_clock})
    )
    si = drain_inst.ins.sync_info
    waits = list(si.on_wait) if si and si.on_wait else []
    if len(waits) > 1:
        si.on_wait = []
        insts = self.nc.cur_bb.bb.instructions
        assert insts[-1] is drain_inst.ins
        insts.pop()
        for w in waits:
            nop = self.nc.sync.nop(nofuse=True)
            nsi = nop.ins.sync_info
            if nsi is None:
                nop.ins.sync_info = mybir.SyncInfo(on_wait=[w], on_update=[])
            else:
                nsi.on_wait = [w]
        insts.append(drain_inst.ins)
    self.nc.all_engine_barrier()
    assert self.sems is not None
    popped = self.nc._tile_sem_poison_stack.pop()
    assert popped is self._sem_poison
    self.nc.clear_and_free_semaphores(list(self.sems.allocated().values()))
    self.nc.all_engine_barrier()


tile.TileContext._drain_and_barrier = _patched_drain_and_barrier


def _split_waits(nc, maxw=1):
    """This walrus build rejects instructions with more than ~1-2 semaphore
    waits; hoist extras onto same-engine nops inserted before the instruction."""
    for f in nc.m.functions:
        for bb in f.blocks:
            insts = bb.instructions
            new = []
            changed = False
            for inst in list(insts):
                si = inst.sync_info
                waits = list(si.on_wait) if si and si.on_wait else []
                if len(waits) > maxw:
                    changed = True
                    si.on_wait = waits[-maxw:]
                    for w in waits[:-maxw]:
                        nop = nc.engines[inst.engine].nop(nofuse=True)
                        cb = nc.cur_bb.bb.instructions
                        assert cb[-1] is nop.ins
                        cb.pop()
                        if nop.ins.sync_info is None:
                            nop.ins.sync_info = mybir.SyncInfo(
                                on_wait=[w], on_update=[])
                        else:
                            nop.ins.sync_info.on_wait = [w]
                        new.append(nop.ins)
                new.append(inst)
            if changed:
                while len(insts):
                    insts.pop()
                for i in new:
                    insts.append(i)


def _pairmul(nc, out_ap, base, rows, engine=None):
    """out[p,i,j] = sum_k A[p,i,k]*B[p,k,j] first half: the multiply.
    base: AP of a [rows, 162] tile region (A | B). Returns nothing."""
    p_ent = list(base.ap[0])
    a_ap = RAP(base.tensor, base.offset, [p_ent, [L, L], [0, L], [1, L]])
    b_ap = RAP(base.tensor, base.offset + L2, [p_ent, [0, L], [1, L], [L, L]])
    eng = engine or nc.vector
    eng.tensor_tensor(out_ap, a_ap, b_ap, op=AluOpType.mult)


# ---------------------------------------------------------------------------
def build_body(tc, reps=1, phases=('load','stats','ln','tr','mm','em','tree')):
    nc = tc.nc
    x_d = nc.dram_tensor("x", [T, H], BF16, kind="ExternalInput").ap()
    w1_d = nc.dram_tensor("w1", [128, KS * MS * 128], BF16,
                          kind="ExternalInput").ap()
    w2_d = nc.dram_tensor("w2", [128, MS * L], BF16, kind="ExternalInput").ap()
    b1_d = nc.dram_tensor("b1", [128, MS], F32, kind="ExternalInput").ap()
    t9_d = nc.dram_tensor("t9", [128, L2], BF16, kind="ExternalInput").ap()
    i81_d = nc.dram_tensor("i81", [1, L2], BF16, kind="ExternalInput").ap()
    oh_d = nc.dram_tensor("oh", [128, NG * 36], F32, kind="ExternalInput").ap()

    om_d = nc.dram_tensor("out_m", [BPC, L2], F32, kind="ExternalOutput").ap()
    oe_d = nc.dram_tensor("out_em0", [1, NG * L], F32,
                          kind="ExternalOutput").ap()
    on_d = nc.dram_tensor("out_ne", [1, NG], F32, kind="ExternalOutput").ap()
    lm_d = nc.dram_tensor("out_lm", [128, 4], F32, kind="ExternalOutput").ap()

    from contextlib import ExitStack
    ctx = tc._build_ctx = ExitStack()
    ctx.__enter__()

    const = ctx.enter_context(tc.tile_pool(name="const", bufs=1))
    xpool = ctx.enter_context(tc.tile_pool(name="xp", bufs=1))
    jpool = ctx.enter_context(tc.tile_pool(name="junk", bufs=2))
    stp = ctx.enter_context(tc.tile_pool(name="stats", bufs=2))
    xnp = ctx.enter_context(tc.tile_pool(name="xn", bufs=3))
    xntp = ctx.enter_context(tc.tile_pool(name="xnt", bufs=2))
    hpool = ctx.enter_context(tc.tile_pool(name="h", bufs=2))
    j3p = ctx.enter_context(tc.tile_pool(name="junk3", bufs=2))
    e4p = ctx.enter_context(tc.tile_pool(name="e4", bufs=2))
    apool = ctx.enter_context(tc.tile_pool(name="abuild", bufs=1))
    c1p = ctx.enter_context(tc.tile_pool(name="c1", bufs=1))
    perp = ctx.enter_context(tc.tile_pool(name="pers", bufs=1))
    lpool = ctx.enter_context(tc.tile_pool(name="lvin", bufs=4))
    ppool = ctx.enter_context(tc.tile_pool(name="prod", bufs=2))
    cpool = ctx.enter_context(tc.tile_pool(name="cout", bufs=4))
    spool = ctx.enter_context(tc.tile_pool(name="scal", bufs=8))

    hppool = ctx.enter_context(tc.tile_pool(name="hp", bufs=3, space="PSUM"))
    empool = ctx.enter_context(tc.tile_pool(name="emp", bufs=2, space="PSUM"))
    nppool = ctx.enter_context(tc.tile_pool(name="np", bufs=1, space="PSUM"))

    dram = ctx.enter_context(tc.tile_pool(name="dram", bufs=1, space="DRAM"))

    # ---- constants to SBUF (one DMA each, host pre-packed)
    w1_sb = const.tile([128, KS * MS * 128], BF16, tag="w1")
    nc.sync.dma_start(w1_sb[:, :], w1_d[:, :])
    w2_sb = const.tile([128, MS * L], BF16, tag="w2")
    nc.sync.dma_start(w2_sb[:, :], w2_d[:, :])
    b1_sb = const.tile([128, MS], F32, tag="b1")
    nc.sync.dma_start(b1_sb[:, :], b1_d[:, :])
    t9_sb = const.tile([128, L2], BF16, tag="t9")
    nc.sync.dma_start(t9_sb[:, :], t9_d[:, :])
    i81_sb = const.tile([1, L2], BF16, tag="i81")
    nc.sync.dma_start(i81_sb[:, :], i81_d[:, :])
    oh_sb = const.tile([128, NG * 36], F32, tag="oh")
    nc.sync.dma_start(oh_sb[:, :], oh_d[:, :])
    ones_sb = const.tile([128, 1], F32, tag="ones")
    nc.gpsimd.memset(ones_sb[:, :], 1.0)
    eps_sb = const.tile([128, 1], F32, tag="eps")
    nc.gpsimd.memset(eps_sb[:, :], EPS)

    acc_all = perp.tile([128, NG], F32, tag="accall")
    em0_all = perp.tile([1, NG * L], F32, tag="em0all")
    lm_all = perp.tile([128, 4], F32, tag="lmall")
    nc.gpsimd.memset(lm_all[:, :], 0.0)

    # SBUF tree ladder: lads[lvl] holds level-lvl output rows (T >> lvl of
    # them), row r at (partition r%128, col-block r//128), 81 wide.
    lads = {lvl: perp.tile([128, max(1, (T >> lvl) // 128) * L2], BF16,
                           tag=f"lad{lvl}", name=f"lad{lvl}")
            for lvl in range(1, NLV)}

    t9q = (t9_sb[0:128, :].rearrange("p (i j) -> p i j", i=L)
           .unsqueeze(1).broadcast_to([128, 2, L, L]))

    for _rep in range(reps):
        _emit_main(tc, nc, locals(), phases)

    ctx.close()


def _emit_main(tc, nc, env, phases=('load','stats','ln','tr','mm','em','tree')):
    g = None  # populated below from env
    (x_d, om_d, oe_d, on_d, lm_d, w1_sb, w2_sb, b1_sb, t9_sb, i81_sb, oh_sb,
     ones_sb, eps_sb, acc_all, em0_all, lm_all, lads, t9q) = (
        env[k] for k in (
            "x_d", "om_d", "oe_d", "on_d", "lm_d", "w1_sb", "w2_sb", "b1_sb",
            "t9_sb", "i81_sb", "oh_sb", "ones_sb", "eps_sb", "acc_all",
            "em0_all", "lm_all", "lads", "t9q"))
    lad1 = lads[1]
    (xpool, jpool, stp, xnp, xntp, hpool, j3p, e4p, apool, c1p, lpool, ppool,
     cpool, spool, hppool, empool, nppool, perp) = (
        env[k] for k in (
            "xpool", "jpool", "stp", "xnp", "xntp", "hpool", "j3p", "e4p",
            "apool", "c1p", "lpool", "ppool", "cpool", "spool", "hppool",
            "empool", "nppool", "perp"))

    # ===== PASS A: load all x, accumulate stats (Act: -mean, Pool: E[x^2])
    x_ts = []
    sx_all = stp.tile([128, 4 * NG], F32, tag="sxall")   # -mean per (g,u)
    q_all = stp.tile([128, 4 * NG], F32, tag="qall")     # E[x^2]
    for g in range(NG):
        x_t = xpool.tile([128, 4, H], BF16, tag=f"x{g}")
        nc.sync.dma_start(
            x_t[:, :, :],
            x_d[g * 512:(g + 1) * 512, :].rearrange("(u p) h -> p u h", u=4))
        x_ts.append(x_t)
        if 'stats' not in phases:
            continue
        for u in range(4):
            c = g * 4 + u
            xh = x_t[:, u, :]
            junk_a = jpool.tile([128, H], BF16, tag="junka")
            nc.scalar.activation(junk_a[:, :], xh, AF.Identity,
                                 scale=-1.0 / H,
                                 accum_out=sx_all[:, c:c + 1])
            junk2 = jpool.tile([128, H], BF16, tag="junk2")
            nc.vector.scalar_tensor_tensor(
                out=junk2[:, :], in0=xh, scalar=1.0 / H, in1=xh,
                op0=AluOpType.mult, op1=AluOpType.mult,
                accum_out=q_all[:, c:c + 1])

    # ===== PASS B: one batched scalar chain for all groups [128, 32]
    if 'stats' in phases:
        NC_ = 4 * NG
        msq = stp.tile([128, NC_], F32, tag="msq")
        nc.vector.tensor_tensor(msq[:, :], sx_all[:, :], sx_all[:, :],
                                op=AluOpType.mult)
        var_t = stp.tile([128, NC_], F32, tag="var")
        nc.vector.tensor_tensor(var_t[:, :], q_all[:, :], msq[:, :],
                                op=AluOpType.subtract)
        sd = stp.tile([128, NC_], F32, tag="sd")
        nc.scalar.activation(sd[:, :], var_t[:, :], AF.Sqrt,
                             bias=eps_sb[:, 0:1])
        rstd = stp.tile([128, NC_], F32, tag="rstd")
        nc.vector.reciprocal(rstd[:, :], sd[:, :])
        nmr = stp.tile([128, NC_], F32, tag="nmr")
        nc.vector.tensor_tensor(nmr[:, :], sx_all[:, :], rstd[:, :],
                                op=AluOpType.mult)

    # ===== PASS C: per group LN + transpose + mm1/gelu + mm2 (Act:
    # Identity+Gelu only - one act table). em results parked in SBUF.
    em_all = perp.tile([128, NG * 36], F32, tag="emall")
    for g in range(NG):
        if 'ln' not in phases:
            break
        xnT = xntp.tile([128, KS * 512], BF16, tag="xnt")
        xnT_v = xnT[:, :].rearrange("p (k t) -> p k t", k=KS)
        for u in range(4):
            c = g * 4 + u
            xn_t = xnp.tile([128, H], BF16, tag="xn")
            nc.scalar.activation(xn_t[:, :], x_ts[g][:, u, :], AF.Identity,
                                 bias=nmr[:, c:c + 1], scale=rstd[:, c:c + 1])
            if 'tr' not in phases:
                continue
            # one fat xbar transpose: [128 tok,1024 h]->[128 h',8 k,128 t]
            nc.sync.dma_start(
                out=xnT_v[:, :, u * 128:(u + 1) * 128],
                in_=xn_t[:, :], transpose=True)
        # ---- mm1 + gelu
        if 'mm' not in phases:
            continue
        h_sb = []
        for m in range(MS):
            hp = hppool.tile([128, 512], F32, tag="hp")
            for k in range(KS):
                nc.tensor.matmul(
                    hp[:, :],
                    w1_sb[:, (k * MS + m) * 128:(k * MS + m + 1) * 128],
                    xnT_v[:, k, :],
                    start=(k == 0), stop=(k == KS - 1))
            h_m = hpool.tile([128, 512], BF16, tag=f"h{m}")
            nc.scalar.activation(h_m[:, :], hp[:, :], AF.Gelu,
                                 bias=b1_sb[:, m:m + 1], scale=1.0)
            h_sb.append(h_m)
        # ---- mm2: emissions, token pairs on partitions q=(a,r): [128,(sp,q,9)]
        # partition q = a*64+r holds tokens (sp*2+a)*128 + 2r + qh
        em_p = empool.tile([128, 36], F32, tag="emp")
        for sp in range(2):
            for qh in range(2):
                for m in range(MS):
                    nc.tensor.matmul(
                        em_p[:, sp * 18 + qh * L: sp * 18 + (qh + 1) * L],
                        h_sb[m][:, :].rearrange(
                            "p (sp a r two) -> p sp two a r",
                            sp=2, a=2, r=64, two=2)[:, sp, qh, :, :],
                        w2_sb[:, m * L:(m + 1) * L],
                        start=(m == 0), stop=(m == MS - 1))
        if 'em' not in phases:
            continue
        # numerator: sum_t em[t, tag_t]
        junk3 = j3p.tile([128, 36], F32, tag="junk3")
        nc.vector.scalar_tensor_tensor(
            out=junk3[:, :], in0=em_p[:, :], scalar=1.0,
            in1=oh_sb[:, g * 36:(g + 1) * 36],
            op0=AluOpType.mult, op1=AluOpType.mult,
            accum_out=acc_all[:, g:g + 1])
        # park emissions in SBUF (em0 slice + exp later)
        nc.vector.tensor_copy(em_all[:, g * 36:(g + 1) * 36], em_p[:, :])

    # ===== PASS D: one Exp for all groups, then all L1 work in waves
    if 'em' in phases and 'mm' in phases and 'ln' in phases:
        # em0: token 0 = partition 0, sp=0, q=0 of each group
        nc.vector.tensor_copy(
            em0_all[0:1, :].rearrange("p (g f) -> p g f", g=NG),
            em_all[0:1, :].rearrange("p (g f) -> p g f", g=NG)[:, :, 0:L])
        E_all = e4p.tile([128, NG * 36], BF16, tag="Eall")
        nc.scalar.activation(E_all[:, :], em_all[:, :], AF.Exp)
        A_ts = []
        for g in range(NG):
            for sp in range(2):
                A_t = apool.tile([128, 2 * L2], BF16, tag=f"A{g}{sp}")
                ev = (E_all[:, g * 36 + sp * 18:g * 36 + (sp + 1) * 18]
                      .rearrange("p (two j) -> p two j", two=2)
                      .unsqueeze(2).broadcast_to([128, 2, L, L]))
                nc.vector.tensor_tensor(
                    A_t[:, :].rearrange("p (two i j) -> p two i j",
                                        two=2, i=L),
                    t9q, ev, op=AluOpType.mult)
                if sp == 0:
                    nc.vector.tensor_copy(A_t[0:1, 0:L2], i81_sb[0:1, :])
                A_ts.append(A_t)
        P_ts = []
        for g in range(NG):
            for sp in range(2):
                P_t = ppool.tile([128, L2 * L], BF16, tag=f"prod{g}{sp}")
                _pairmul(nc, P_t[:, :].rearrange("p (i j k) -> p i j k",
                                                 i=L, j=L),
                         A_ts[g * 2 + sp][:, :], 128, engine=nc.gpsimd)
                P_ts.append(P_t)
        # L1 reduce writes straight into the SBUF ladder: row g*256+sp*128+q
        # lives at (partition q, block g*2+sp) -- no store DMA needed.
        for g in range(NG):
            for sp in range(2):
                blk = g * 2 + sp
                with nc.allow_low_precision("fp32 ALU, single output round"):
                    nc.vector.reduce_sum(
                        lad1[:, blk * L2:(blk + 1) * L2],
                        P_ts[g * 2 + sp][:, :].rearrange(
                            "p (f k) -> p f k", k=L),
                        axis=AX.X)

    # numerator partition-sum via ones-matmul
    if 'em' in phases:
        np_p = nppool.tile([1, NG], F32, tag="npp")
        nc.tensor.matmul(np_p[:, :], ones_sb[:, :], acc_all[:, :])
        ne_sb = spool.tile([1, NG], F32, tag="ne")
        nc.vector.tensor_copy(ne_sb[:, :], np_p[:, :])
        nc.sync.dma_start(on_d[0:1, :], ne_sb[:, :])
        nc.sync.dma_start(oe_d[0:1, :], em0_all[0:1, :])

    # ================= CRF tree levels 2..9 (SBUF-resident ladder) ========
    # lad[lvl] holds level-lvl output: row r at (partition r%128, block
    # r//128). The pair-gather to the next level is 2-4 small SBUF->SBUF
    # DMAs with stride-2 partitions, split across the SP and Act queues.
    if 'tree' not in phases:
        return
    for lvl in range(2, NLV + 1):
        rows_in = T >> (lvl - 1)
        rows_out = T >> lvl
        nt = (rows_out + 127) // 128          # tiles this level
        rows_t = min(128, rows_out)           # rows per tile
        src = lads[lvl - 1]
        nb_in = max(1, rows_in // 128)
        # gather: in_t[p, ti, d*81:(d+1)*81] <- src row 256*ti + 2p + d
        in_t = lpool.tile([128, nt * 2 * L2], BF16, tag="lvin")
        if rows_in >= 256:
            srcv = src[:, :].rearrange("p (t2 two f) -> p t2 two f",
                                       two=2, f=L2)
            dstv = in_t[:, :].rearrange("p (t d f) -> p t d f", d=2, f=L2)
            for d in range(2):
                for ph in range(2):
                    eng = nc.sync if (d + ph) % 2 == 0 else nc.scalar
                    eng.dma_start(
                        dstv[64 * ph:64 * (ph + 1), :, d, :],
                        srcv[d:128:2, :, ph, :])
        else:
            cnt = rows_in // 2
            for d in range(2):
                eng = nc.sync if d == 0 else nc.scalar
                eng.dma_start(in_t[0:cnt, d * L2:(d + 1) * L2],
                              src[d:rows_in:2, 0:L2])
        for ti in range(nt):
            P_t = ppool.tile([128, L2 * L], BF16, tag="prod")
            base = in_t[:rows_t, ti * 2 * L2:(ti + 1) * 2 * L2]
            _pairmul(nc, P_t[:rows_t, :].rearrange(
                "p (i j k) -> p i j k", i=L, j=L), base, rows_t,
                engine=(nc.gpsimd if ti % 2 == 0 else nc.vector))
            if lvl not in RESCALE_LEVELS:
                with nc.allow_low_precision("fp32 ALU, single round"):
                    nc.vector.reduce_sum(
                        lads[lvl][:rows_t, ti * L2:(ti + 1) * L2],
                        P_t[:rows_t, :].rearrange("p (f k) -> p f k", k=L),
                        axis=AX.X)
                continue
            C_t = cpool.tile([128, L2], F32, tag="cout")
            nc.vector.reduce_sum(
                C_t[:rows_t, :],
                P_t[:rows_t, :].rearrange("p (f k) -> p f k", k=L),
                axis=AX.X)
            mx = spool.tile([128, 1], F32, tag="mx")
            nc.vector.reduce_max(mx[:rows_t, :], C_t[:rows_t, :], axis=AX.X)
            rmx = spool.tile([128, 1], F32, tag="rmx")
            nc.vector.reciprocal(rmx[:rows_t, :], mx[:rows_t, :])
            col, rr0 = LM_SLOTS[lvl][ti]
            nc.scalar.activation(lm_all[rr0:rr0 + rows_t, col:col + 1],
                                 mx[:rows_t, :], AF.Ln)
            if lvl < NLV:
                nc.vector.tensor_scalar_mul(
                    lads[lvl][:rows_t, ti * L2:(ti + 1) * L2],
                    C_t[:rows_t, :], rmx[:rows_t, 0:1])
            else:
                Cf = cpool.tile([128, L2], F32, tag="cfin")
                nc.vector.tensor_scalar_mul(Cf[:rows_t, :], C_t[:rows_t, :],
                                            rmx[:rows_t, 0:1])
                nc.sync.dma_start(om_d[:, :], Cf[:BPC, :])

    nc.sync.dma_start(lm_d[:, :], lm_all[:, :])


def build_program(reps=1, phases=('load','stats','ln','tr','mm','em','tree')):
    nc = bass.Bass("TRN2", target_bir_lowering=False, debug=False)
    with tile.TileContext(nc) as tc:
        build_body(tc, reps=reps, phases=phases)
    _split_waits(nc)
    return nc


# ---------------------------------------------------------------------------
_CACHED = {}


def _get_program():
    if "nc" not in _CACHED:
        _CACHED["nc"] = build_program()
    return _CACHED["nc"]


def _host_prep(hidden_states, ln_gamma, ln_beta, W1, b1, W2, b2,
               start_trans, end_trans, trans, labels, attention_mask):
    x = np.ascontiguousarray(hidden_states, np.float32).reshape(B * S, H)
    tg = np.asarray(labels)
    W1p = (np.asarray(ln_gamma)[:, None] * np.asarray(W1)).astype(np.float32)
    b1p = (np.asarray(b1) + np.asarray(ln_beta) @ np.asarray(W1)).astype(
        np.float32)
    # w1 packed: [128, (k*MS+m)*128 + c] = W1p[k*128+p, m*128+c]
    w1t = np.ascontiguousarray(
        W1p.reshape(KS, 128, MS, 128).transpose(1, 0, 2, 3).reshape(
            128, KS * MS * 128)).astype(nbf16)
    # w2 packed: [128, m*9+l] = W2[m*128+p, l]
    w2t = np.ascontiguousarray(
        np.asarray(W2).reshape(MS, 128, L).transpose(1, 0, 2).reshape(
            128, MS * L)).astype(nbf16)
    T9b2 = np.exp(np.asarray(trans) + np.asarray(b2)[None, :])
    t9b = np.broadcast_to(T9b2.reshape(1, L2), (128, L2)).astype(nbf16)
    i81 = np.eye(L, dtype=np.float32).reshape(1, L2).astype(nbf16)
    b1_tile = np.ascontiguousarray(b1p.reshape(MS, 128).T, np.float32)

    oh_full = np.zeros((B * S, L), np.float32)
    oh_full[np.arange(B * S), tg.reshape(-1)] = 1.0

    num_table = (np.asarray(start_trans)[tg[:, 0]]
                 + np.asarray(trans)[tg[:, :-1], tg[:, 1:]].sum(1)
                 + np.asarray(end_trans)[tg[:, -1]]
                 + np.asarray(b2)[tg].sum(1)).astype(np.float64)

    xb = x.astype(nbf16)
    in_maps = []
    for c in range(NCORES):
        xc = np.ascontiguousarray(xb[c * T:(c + 1) * T])
        # oh layout: [a*64+r, g*36 + sp*18 + qh*9 + l]
        #   = onehot[g*512 + (sp*2+a)*128 + 2r + qh, l]
        ohc = oh_full[c * T:(c + 1) * T].reshape(NG, 2, 2, 64, 2, L)
        ohc = np.ascontiguousarray(
            ohc.transpose(2, 3, 0, 1, 4, 5).reshape(128, NG * 36))
        in_maps.append({
            "x": xc, "w1": w1t, "w2": w2t, "b1": b1_tile,
            "t9": t9b, "i81": i81, "oh": ohc,
        })
    return in_maps, num_table


def _scale_from_lm(lm):
    """Sum per-example rescale logs from the packed lm_all [128, 4].

    Rescales happen at levels 4 (256 rows, 2 tiles in cols 0/1), 7 (32
    rows, col 2) and 9 (8 rows, col 3)."""
    sf = np.zeros(BPC, np.float64)
    for i in range(BPC):
        # L4: 256 rows, 32/example: col i//4, rows 32*(i%4)..+32
        sf[i] += lm[32 * (i % 4):32 * (i % 4) + 32, i // 4].sum()
        # L7: 32 rows, 4/example: col 2 rows 4i..4i+4
        sf[i] += lm[4 * i:4 * i + 4, 2].sum()
        # L9: 8 rows, 1/example: col 3 row i
        sf[i] += lm[i, 3]
    return sf


def _assemble(results, num_table, start_trans, end_trans, b2):
    start_trans = np.asarray(start_trans, np.float64)
    end_trans = np.asarray(end_trans, np.float64)
    b2 = np.asarray(b2, np.float64)
    llh = np.zeros(B, np.float64)
    for c in range(NCORES):
        r = results[c]
        Mf = np.asarray(r["out_m"], np.float64)
        Sf = _scale_from_lm(np.asarray(r["out_lm"], np.float64))
        em0 = np.asarray(r["out_em0"], np.float64).reshape(BPC, L)
        ne = np.asarray(r["out_ne"], np.float64)[0]
        logM = np.log(Mf).reshape(BPC, L, L) + Sf[:, None, None]
        score0 = start_trans[None, :] + em0 + b2[None, :]
        zz = score0[:, :, None] + logM + end_trans[None, None, :]
        mz = zz.max((1, 2), keepdims=True)
        denom = np.log(np.exp(zz - mz).sum((1, 2))) + mz[:, 0, 0]
        num = num_table[c * BPC:(c + 1) * BPC] + ne
        llh[c * BPC:(c + 1) * BPC] = num - denom
    return np.float32(-llh.mean())


def _reference_numpy(hidden_states, ln_gamma, ln_beta, W1, b1, W2, b2,
                     start_trans, end_trans, trans, labels, attention_mask):
    """Exact fallback (general mask/labels), pure numpy."""
    from scipy.special import erf
    x = np.asarray(hidden_states, np.float32)
    mu = x.mean(-1, keepdims=True)
    var = ((x - mu) ** 2).mean(-1, keepdims=True)
    xn = (x - mu) / np.sqrt(var + EPS) * ln_gamma + ln_beta
    hpre = xn @ W1 + b1
    h = 0.5 * hpre * (1 + erf(hpre / np.sqrt(2.0)))
    em = h @ W2 + b2
    labels = np.asarray(labels)
    mask = (labels != -100) & (np.asarray(attention_mask) == 1)
    mask[:, 0] = True
    tags = np.where(labels == -100, 0, labels)
    em_t = em.transpose(1, 0, 2).astype(np.float64)
    m = mask.T
    tg = tags.T
    mf = m.astype(np.float64)
    bar = np.arange(em_t.shape[1])
    em_sc = np.take_along_axis(em_t, tg[:, :, None], 2)[:, :, 0]
    pair = np.asarray(trans)[tg[:-1], tg[1:]]
    num = (np.asarray(start_trans)[tg[0]] + em_sc[0]
           + ((pair + em_sc[1:]) * mf[1:]).sum(0))
    seq_ends = m.astype(np.int64).sum(0) - 1
    num = num + np.asarray(end_trans)[tg[seq_ends, bar], ]
    score = np.asarray(start_trans)[None, :] + em_t[0]
    for i in range(1, em_t.shape[0]):
        z = score[:, :, None] + np.asarray(trans)[None] + em_t[i][:, None, :]
        zm = z.max(1, keepdims=True)
        nxt = np.log(np.exp(z - zm).sum(1)) + zm[:, 0, :]
        score = np.where(m[i][:, None], nxt, score)
    z = score + np.asarray(end_trans)[None, :]
    zm = z.max(1, keepdims=True)
    denom = np.log(np.exp(z - zm).sum(1)) + zm[:, 0]
    return np.float32(-(num - denom).mean())


def kernel(**inputs):
    labels = np.asarray(inputs["labels"])
    am = np.asarray(inputs["attention_mask"])
    if not ((am == 1).all() and (labels >= 0).all() and (labels < L).all()):
        return _reference_numpy(**inputs)

    from concourse.bass_utils import run_bass_kernel_spmd
    nc = _get_program()
    in_maps, num_table = _host_prep(**inputs)
    res = run_bass_kernel_spmd(nc, in_maps, list(range(NCORES)))
    return _assemble(res.results, num_table,
                     inputs["start_trans"], inputs["end_trans"], inputs["b2"])

